# revision 1
# baseline (speedup 1.0000x reference)
"""Multi-head attention (B=2, S=2048, D=1024, H=16) on 8 trn2 NeuronCores.

Sharding: data-parallel over batch (2 groups of 4 cores), tensor-parallel over
heads within a group (4 heads/core).  Each core computes a partial output
(its heads' contribution through its W_o column shard); the host sums the 4
partials per batch element.

Per-core kernel layout choices:
  - q, k are produced TRANSPOSED ([d_local, s], head dim on partitions) so the
    scores matmul s^T[s_k, s_q] needs no transpose: lhsT = kT tile, rhs = qT.
    Dk=64 < 128, so the two heads of a pair are row-packed into the PE array
    via tile_position=(0,0)/(64,0) and run concurrently.
  - v is produced in natural layout [s, d] with a fused ones-column so the AV
    matmul (lhsT = v_aug stationary [128, 65], rhs = exp tile) emits both the
    attention output (rows 0:64, transposed) and the softmax denominator
    (row 64) in one accumulation chain.  Softmax skips max-subtraction
    (scores ~ N(0,1) after the 1/8 scale; exp cannot overflow in fp32).
  - exp runs on ACT directly from PSUM in one [128, 2048]-wide op per k-tile
    (both heads), with the 1/sqrt(64) scale folded into the activation.
"""

import os
from contextlib import ExitStack

import numpy as np

B = 2
S = 2048
DM = 1024
H = 16
DK = 64
P = 128
HC = 4            # heads per core
DO = HC * DK      # 256: local output dim of q/k/v projections
DI_T = DM // P    # 8 contraction tiles for projections
S_T = S // P      # 16
QC = 1024         # s_q chunk processed per attention block
N_QC = S // QC

MM_BF16 = True    # matmul inputs in fp16 (fp32 PSUM accumulation everywhere)

_PROGRAM = None


def _build_program():
    import concourse.mybir as mybir
    import concourse.tile as tile
    from concourse import bacc

    f32 = mybir.dt.float32
    mmdt = mybir.dt.float16 if MM_BF16 else f32
    nc = bacc.Bacc("TRN2", target_bir_lowering=False, debug=False)

    qt_d = nc.dram_tensor("QT", [DM, S], mmdt, kind="ExternalInput").ap()
    kt_d = nc.dram_tensor("KT", [DM, S], mmdt, kind="ExternalInput").ap()
    vt_d = nc.dram_tensor("VT", [DM, S], mmdt, kind="ExternalInput").ap()
    wqt_d = nc.dram_tensor("WQT", [DM, DO], mmdt, kind="ExternalInput").ap()
    wkt_d = nc.dram_tensor("WKT", [DM, DO], mmdt, kind="ExternalInput").ap()
    wvt_d = nc.dram_tensor("WVT", [DM, DO], mmdt, kind="ExternalInput").ap()
    wot_d = nc.dram_tensor("WOT", [DO, DM], mmdt, kind="ExternalInput").ap()
    out_d = nc.dram_tensor("OUT", [S, DM], f32, kind="ExternalOutput").ap()

    with tile.TileContext(nc) as tc, ExitStack() as ctx:
        _emit(ctx, tc, qt_d, kt_d, vt_d, wqt_d, wkt_d, wvt_d, wot_d, out_d)
    nc.compile()
    return nc


def _emit(ctx, tc, qt_d, kt_d, vt_d, wqt_d, wkt_d, wvt_d, wot_d, out_d):
    import concourse.mybir as mybir

    nc = tc.nc
    f32 = mybir.dt.float32
    mmdt = mybir.dt.float16 if MM_BF16 else f32
    Exp = mybir.ActivationFunctionType.Exp

    consts = ctx.enter_context(tc.tile_pool(name="consts", bufs=1))
    exp_pool = ctx.enter_context(tc.tile_pool(name="exp", bufs=3))
    smalls = ctx.enter_context(tc.tile_pool(name="smalls", bufs=2))
    ostage = ctx.enter_context(tc.tile_pool(name="ostage", bufs=2))

    # persistent SBUF tensors; the full inputs are staged in SBUF so that all
    # matmuls are DMA-independent (keeps the PE dense -> HAM stays at 8/8)
    qt_sb = consts.tile([P, DI_T, S], mmdt, tag="qt")     # staged Q^T
    kt_sb = consts.tile([P, DI_T, S], mmdt, tag="kt")
    vt_sb = consts.tile([P, DI_T, S], mmdt, tag="vt")
    wq_sb = consts.tile([P, DI_T, DO], mmdt, tag="wq")
    wk_sb = consts.tile([P, DI_T, DO], mmdt, tag="wk")
    wv_sb = consts.tile([P, DI_T, DO], mmdt, tag="wv")
    wo_sb = consts.tile([P, 2, DM], mmdt, tag="wo")
    qT_sb = consts.tile([P, 2, S], mmdt, tag="qT")    # [p, ot, s]; o_local = ot*128+p
    kT_sb = consts.tile([P, 2, S], mmdt, tag="kT")
    vab_sb = consts.tile([P, S_T, HC, DK + 1], mmdt, tag="vab")
    attnT_sb = consts.tile([P, 2, S], mmdt, tag="attnT")
    stage_sb = consts.tile([64, 2, S], mmdt, tag="oddstage")
    ones_sb = consts.tile([1, 64], f32, tag="ones")
    warm_sb = consts.tile([P, 512], mmdt, tag="warm")

    # weights first (small), then K (needed first), V, Q
    nc.sync.dma_start(wk_sb[:], wkt_d.rearrange("(t p) o -> p t o", p=P))
    nc.sync.dma_start(wv_sb[:], wvt_d.rearrange("(t p) o -> p t o", p=P))
    nc.sync.dma_start(wq_sb[:], wqt_d.rearrange("(t p) o -> p t o", p=P))
    nc.sync.dma_start(wo_sb[:], wot_d.rearrange("(t p) o -> p t o", p=P))
    nc.sync.dma_start(kt_sb[:], kt_d.rearrange("(t p) s -> p t s", p=P))
    nc.sync.dma_start(vt_sb[:], vt_d.rearrange("(t p) s -> p t s", p=P))
    nc.sync.dma_start(qt_sb[:], qt_d.rearrange("(t p) s -> p t s", p=P))
    nc.vector.memset(vab_sb[:, :, :, DK : DK + 1], 1.0)
    nc.vector.memset(ones_sb[:], 1.0)
    nc.vector.memset(warm_sb[:], 0.0)

    psum_proj = ctx.enter_context(tc.tile_pool(name="psum_proj", bufs=2, space="PSUM"))
    psum_s_pool = ctx.enter_context(tc.tile_pool(name="psum_s", bufs=2, space="PSUM"))
    psum_av_pool = ctx.enter_context(tc.tile_pool(name="psum_av", bufs=2, space="PSUM"))

    # PE warmup: dense matmuls (no data deps) flip the HAM clock gate to 8/8
    # and cover the input-DMA latency.
    for i in range(40):
        wp = psum_proj.tile([P, 512], f32, tag="proj", name="warmps")
        nc.tensor.matmul(wp[:], warm_sb[:, 0:P], warm_sb[:], start=True, stop=True)

    def proj_qk(src_sb, wsb, dst, sc):
        # one 512-wide s-chunk of a transposed projection (both o-tiles)
        for ot in range(2):
            pp = psum_proj.tile([P, 512], f32, tag="proj", name="pp")
            for t in range(DI_T):
                nc.tensor.matmul(
                    pp[:],
                    wsb[:, t, ot * P : (ot + 1) * P],
                    src_sb[:, t, sc * 512 : (sc + 1) * 512],
                    start=(t == 0),
                    stop=(t == DI_T - 1),
                )
            nc.vector.tensor_copy(dst[:, ot, sc * 512 : (sc + 1) * 512], pp[:])

    # K projection (transposed layout)
    for sc in range(4):
        proj_qk(kt_sb, wk_sb, kT_sb, sc)

    # V projection (natural layout, into vab; ones column preset)
    for st in range(S_T):
        pv = psum_proj.tile([P, 256], f32, tag="proj", name="pv")
        for t in range(DI_T):
            nc.tensor.matmul(
                pv[:],
                vt_sb[:, t, st * P : (st + 1) * P],
                wv_sb[:, t, :],
                start=(t == 0),
                stop=(t == DI_T - 1),
            )
        nc.vector.tensor_copy(
            vab_sb[:, st, :, 0:DK],
            pv[:].rearrange("p (h d) -> p h d", d=DK),
        )

    # attention + q-projection + output projection, interleaved per 512-chunk
    NCH = S // 512
    for ch in range(NCH):
        q0 = ch * 512
        proj_qk(qt_sb, wq_sb, qT_sb, ch)
        for hp in range(2):
            av = [
                psum_av_pool.tile([P, 512], f32, tag="av", name=f"av{j}")
                for j in range(2)
            ]
            ps_tiles = {}

            def scores(t, hp=hp, q0=q0, ps_tiles=ps_tiles):
                ps_s = psum_s_pool.tile([P, 2, 512], f32, tag="scores", name="ps_s")
                ps_tiles[t] = ps_s
                for j in range(2):
                    hb = j * 64
                    nc.tensor.matmul(
                        ps_s[:, j, :],
                        kT_sb[hb : hb + 64, hp, t * P : (t + 1) * P],
                        qT_sb[hb : hb + 64, hp, q0 : q0 + 512],
                        start=True,
                        stop=True,
                        tile_position=(hb, 0),
                    )

            ex_tiles = {}

            def expop(t, ex_tiles=ex_tiles, ps_tiles=ps_tiles):
                ex = exp_pool.tile([P, 2, 512], mmdt, tag="exp", name="ex")
                ex_tiles[t] = ex
                nc.scalar.activation(ex[:], ps_tiles.pop(t)[:], Exp, scale=0.125)

            def avop(t, hp=hp, av=av, ex_tiles=ex_tiles):
                ex = ex_tiles.pop(t)
                for j in range(2):
                    nc.tensor.matmul(
                        av[j][0 : DK + 1, :],
                        vab_sb[:, t, 2 * hp + j, :],
                        ex[:, j, :],
                        start=(t == 0),
                        stop=(t == S_T - 1),
                    )

            # software-pipelined emission: PE always has independent work
            # (scores of t+1, AV of t-1) while ACT runs exp(t)
            scores(0)
            expop(0)
            for t in range(1, S_T):
                scores(t)
                avop(t - 1)
                expop(t)
            avop(S_T - 1)

            # epilogue: divide by the softmax denominators (row DK of av).
            # Broadcast each denom row across 64 partitions with a SWDGE DMA.
            den_b = psum_s_pool.tile([64, 2, 512], f32, tag="scores", name="den_b")
            rec = []
            for j in range(2):
                den_row = smalls.tile([1, 512], f32, tag="den", name=f"den{j}")
                nc.vector.tensor_copy(den_row[:], av[j][DK : DK + 1, :])
                nc.tensor.matmul(
                    den_b[:, j, :], ones_sb[:], den_row[:], start=True, stop=True
                )
                rec_b = smalls.tile([64, 512], f32, tag="recb", name=f"rec{j}")
                nc.vector.reciprocal_approx_fast(rec_b[:], den_b[:, j, :])
                rec.append(rec_b)
            for j in range(2):
                lh = 2 * hp + j
                if lh % 2 == 0:
                    nc.vector.tensor_mul(
                        attnT_sb[0:64, lh // 2, q0 : q0 + 512],
                        av[j][0:DK, :],
                        rec[j][:],
                    )
                else:
                    nc.vector.tensor_mul(
                        stage_sb[:, lh // 2, q0 : q0 + 512],
                        av[j][0:DK, :],
                        rec[j][:],
                    )
                    nc.sync.dma_start(
                        attnT_sb[64:128, lh // 2, q0 : q0 + 512],
                        stage_sb[:, lh // 2, q0 : q0 + 512],
                    )
        # output projection for this chunk's 4 row-tiles
        for si in range(4):
            st = ch * 4 + si
            po = [
                psum_proj.tile([P, 512], f32, tag="proj", name=f"po{c}")
                for c in range(2)
            ]
            for col in range(2):
                for ot in range(2):
                    nc.tensor.matmul(
                        po[col][:],
                        attnT_sb[:, ot, st * P : (st + 1) * P],
                        wo_sb[:, ot, col * 512 : (col + 1) * 512],
                        start=(ot == 0),
                        stop=(ot == 1),
                    )
            ob = ostage.tile([P, DM], f32, tag="ostage")
            for col in range(2):
                nc.vector.tensor_copy(ob[:, col * 512 : (col + 1) * 512], po[col][:])
            nc.sync.dma_start(out_d[st * P : (st + 1) * P, :], ob[:])


def _get_program():
    global _PROGRAM
    if _PROGRAM is None:
        _PROGRAM = _build_program()
    return _PROGRAM


def make_in_maps(Q, K, V, W_q, W_k, W_v, W_o):
    """Per-core input dicts: core c -> batch c//4, heads (c%4)*4 ... +4."""
    mmdt = np.float16 if MM_BF16 else np.float32
    in_maps = []
    for c in range(8):
        b, g = c // 4, c % 4
        sl = slice(g * DO, (g + 1) * DO)
        in_maps.append(
            {
                "QT": np.ascontiguousarray(Q[b].T).astype(mmdt),
                "KT": np.ascontiguousarray(K[b].T).astype(mmdt),
                "VT": np.ascontiguousarray(V[b].T).astype(mmdt),
                "WQT": np.ascontiguousarray(W_q[sl, :].T).astype(mmdt),
                "WKT": np.ascontiguousarray(W_k[sl, :].T).astype(mmdt),
                "WVT": np.ascontiguousarray(W_v[sl, :].T).astype(mmdt),
                "WOT": np.ascontiguousarray(W_o[:, sl].T).astype(mmdt),
            }
        )
    return in_maps


def combine_outputs(outs):
    """outs: list of 8 [S, DM] partials -> [B, S, DM]."""
    return np.stack(
        [
            outs[0] + outs[1] + outs[2] + outs[3],
            outs[4] + outs[5] + outs[6] + outs[7],
        ]
    ).astype(np.float32)


def kernel(Q, K, V, W_q, W_k, W_v, W_o):
    from concourse.bass_utils import run_bass_kernel_spmd

    Q = np.asarray(Q)
    K = np.asarray(K)
    V = np.asarray(V)
    nc = _get_program()
    in_maps = make_in_maps(Q, K, V, np.asarray(W_q), np.asarray(W_k), np.asarray(W_v), np.asarray(W_o))
    res = run_bass_kernel_spmd(nc, in_maps, core_ids=list(range(8)))
    return combine_outputs([res.results[c]["OUT"] for c in range(8)])



# revision 3
# speedup vs baseline: 1.2903x; 1.2903x over previous
"""Multi-head attention (B=2, S=2048, D=1024, H=16) on 8 trn2 NeuronCores.

Sharding: data-parallel over batch (2 groups of 4 cores), tensor-parallel over
heads within a group (4 heads/core).  Each core computes a partial output
(its heads' contribution through its W_o column shard); the host sums the 4
partials per batch element.

Schedule (v2): the kernel is ACT-bound (exp of 4x2048x2048 scores = ~142us at
1 elem/cycle/lane @1.2GHz), so everything else is scheduled around keeping the
exp stream dense:
  - input DMAs are chunked by s-512 so the K/Q projections for the first
    attention block start ~7us in instead of waiting for whole-tensor DMAs.
  - all remaining projection work (V-proj, later K/Q-proj chunks, output
    projection of finished chunks) is drip-fed as background PE work INSIDE
    the attention t-loops, filling the PE slack under each exp instruction
    instead of draining ACT between blocks.
  - the softmax-denominator broadcast matmul runs in fp16 (1-pass) instead of
    fp32 (2-pass), and a dummy exp warms the ACT table load off the
    critical path.

Per-core kernel layout (unchanged from v1):
  - q, k produced TRANSPOSED ([d_local, s]); scores lhsT = kT tile, rhs = qT,
    head pairs row-packed via tile_position (0,0)/(64,0).
  - v in natural layout with a fused ones-column so the AV matmul emits the
    attention output and the softmax denominator in one accumulation chain.
  - softmax skips max-subtraction (scores ~ N(0,1); fp32 exp cannot overflow).
"""

import os
from contextlib import ExitStack

import numpy as np

B = 2
S = 2048
DM = 1024
H = 16
DK = 64
P = 128
HC = 4            # heads per core
DO = HC * DK      # 256: local output dim of q/k/v projections
DI_T = DM // P    # 8 contraction tiles for projections
S_T = S // P      # 16
N_SC = 4          # s-chunks of 512

MM_BF16 = True    # matmul inputs in fp16 (fp32 PSUM accumulation everywhere)

_PROGRAM = None


def _build_program():
    import concourse.mybir as mybir
    import concourse.tile as tile
    from concourse import bacc

    f32 = mybir.dt.float32
    mmdt = mybir.dt.float16 if MM_BF16 else f32
    nc = bacc.Bacc("TRN2", target_bir_lowering=False, debug=False)

    qt_d = nc.dram_tensor("QT", [DM, S], mmdt, kind="ExternalInput").ap()
    kt_d = nc.dram_tensor("KT", [DM, S], mmdt, kind="ExternalInput").ap()
    vt_d = nc.dram_tensor("VT", [DM, S], mmdt, kind="ExternalInput").ap()
    wqt_d = nc.dram_tensor("WQT", [DM, DO], mmdt, kind="ExternalInput").ap()
    wkt_d = nc.dram_tensor("WKT", [DM, DO], mmdt, kind="ExternalInput").ap()
    wvt_d = nc.dram_tensor("WVT", [DM, DO], mmdt, kind="ExternalInput").ap()
    wot_d = nc.dram_tensor("WOT", [DO, DM], mmdt, kind="ExternalInput").ap()
    out_d = nc.dram_tensor("OUT", [S, DM], f32, kind="ExternalOutput").ap()

    with tile.TileContext(nc) as tc, ExitStack() as ctx:
        _emit(ctx, tc, qt_d, kt_d, vt_d, wqt_d, wkt_d, wvt_d, wot_d, out_d)
    nc.compile()
    return nc


def _emit(ctx, tc, qt_d, kt_d, vt_d, wqt_d, wkt_d, wvt_d, wot_d, out_d):
    import concourse.mybir as mybir

    nc = tc.nc
    f32 = mybir.dt.float32
    mmdt = mybir.dt.float16 if MM_BF16 else f32
    Exp = mybir.ActivationFunctionType.Exp

    consts = ctx.enter_context(tc.tile_pool(name="consts", bufs=1))
    exp_pool = ctx.enter_context(tc.tile_pool(name="exp", bufs=3))
    smalls = ctx.enter_context(tc.tile_pool(name="smalls", bufs=2))
    ostage = ctx.enter_context(tc.tile_pool(name="ostage", bufs=2))

    qt_sb = consts.tile([P, DI_T, S], mmdt, tag="qt")     # staged Q^T
    kt_sb = consts.tile([P, DI_T, S], mmdt, tag="kt")
    vt_sb = consts.tile([P, DI_T, S], mmdt, tag="vt")
    wq_sb = consts.tile([P, DI_T, DO], mmdt, tag="wq")
    wk_sb = consts.tile([P, DI_T, DO], mmdt, tag="wk")
    wv_sb = consts.tile([P, DI_T, DO], mmdt, tag="wv")
    wo_sb = consts.tile([P, 2, DM], mmdt, tag="wo")
    qT_sb = consts.tile([P, 2, S], mmdt, tag="qT")    # [p, ot, s]; o_local = ot*128+p
    kT_sb = consts.tile([P, 2, S], mmdt, tag="kT")
    vab_sb = consts.tile([P, S_T, HC, DK + 1], mmdt, tag="vab")
    attnT_sb = consts.tile([P, 2, S], mmdt, tag="attnT")
    stage_sb = consts.tile([64, 2, S], mmdt, tag="oddstage")
    ones_sb = consts.tile([1, 64], mmdt, tag="ones")
    warm_sb = consts.tile([P, 512], mmdt, tag="warm")

    # chunked input DMAs, in consumption-priority order.  each chunk is
    # [p, t, 512] (1KB per partition-line).  kt0/qt0 gate the first block.
    kt_r = kt_d.rearrange("(t p) s -> p t s", p=P)
    qt_r = qt_d.rearrange("(t p) s -> p t s", p=P)
    vt_r = vt_d.rearrange("(t p) s -> p t s", p=P)

    def in_chunk(dst, src, sc):
        nc.sync.dma_start(
            dst[:, :, sc * 512 : (sc + 1) * 512], src[:, :, sc * 512 : (sc + 1) * 512]
        )

    nc.sync.dma_start(wk_sb[:], wkt_d.rearrange("(t p) o -> p t o", p=P))
    in_chunk(kt_sb, kt_r, 0)
    nc.sync.dma_start(wq_sb[:], wqt_d.rearrange("(t p) o -> p t o", p=P))
    in_chunk(qt_sb, qt_r, 0)
    in_chunk(kt_sb, kt_r, 1)
    nc.sync.dma_start(wv_sb[:], wvt_d.rearrange("(t p) o -> p t o", p=P))
    in_chunk(vt_sb, vt_r, 0)
    in_chunk(kt_sb, kt_r, 2)
    in_chunk(kt_sb, kt_r, 3)
    in_chunk(vt_sb, vt_r, 1)
    in_chunk(vt_sb, vt_r, 2)
    in_chunk(vt_sb, vt_r, 3)
    in_chunk(qt_sb, qt_r, 1)
    in_chunk(qt_sb, qt_r, 2)
    in_chunk(qt_sb, qt_r, 3)
    nc.sync.dma_start(wo_sb[:], wot_d.rearrange("(t p) o -> p t o", p=P))

    nc.vector.memset(vab_sb[:, :, :, DK : DK + 1], 1.0)
    nc.vector.memset(ones_sb[:], 1.0)
    nc.vector.memset(warm_sb[:], 0.0)

    psum_proj = ctx.enter_context(tc.tile_pool(name="psum_proj", bufs=2, space="PSUM"))
    psum_s_pool = ctx.enter_context(tc.tile_pool(name="psum_s", bufs=2, space="PSUM"))
    psum_av_pool = ctx.enter_context(tc.tile_pool(name="psum_av", bufs=2, space="PSUM"))

    # hoist the ~2.7us exp table load off the critical path (ACT is idle here)
    dummy_ex = smalls.tile([1, 8], f32, tag="dummyex")
    nc.scalar.activation(dummy_ex[:], warm_sb[0:1, 0:8], Exp)

    # PE warmup: flips the HAM clock gate to 8/8 and covers input-DMA latency
    for i in range(16):
        wp = psum_proj.tile([P, 512], f32, tag="proj", name="warmps")
        nc.tensor.matmul(wp[:], warm_sb[:, 0:P], warm_sb[:], start=True, stop=True)

    # ---- background PE work units -------------------------------------
    def kp_unit(sc, ot, src_sb=None, wsb=None, dst=None):
        # one 512-wide s-chunk, one o-tile of a transposed projection
        src_sb = kt_sb if src_sb is None else src_sb
        wsb = wk_sb if wsb is None else wsb
        dst = kT_sb if dst is None else dst
        pp = psum_proj.tile([P, 512], f32, tag="proj", name="pp")
        for t in range(DI_T):
            nc.tensor.matmul(
                pp[:],
                wsb[:, t, ot * P : (ot + 1) * P],
                src_sb[:, t, sc * 512 : (sc + 1) * 512],
                start=(t == 0),
                stop=(t == DI_T - 1),
            )
        nc.vector.tensor_copy(dst[:, ot, sc * 512 : (sc + 1) * 512], pp[:])

    def qp_unit(sc, ot):
        kp_unit(sc, ot, src_sb=qt_sb, wsb=wq_sb, dst=qT_sb)

    def vp_unit(st):
        # V projection s-tile (natural layout, into vab; ones column preset)
        pv = psum_proj.tile([P, 256], f32, tag="proj", name="pv")
        for t in range(DI_T):
            nc.tensor.matmul(
                pv[:],
                vt_sb[:, t, st * P : (st + 1) * P],
                wv_sb[:, t, :],
                start=(t == 0),
                stop=(t == DI_T - 1),
            )
        nc.vector.tensor_copy(
            vab_sb[:, st, :, 0:DK],
            pv[:].rearrange("p (h d) -> p h d", d=DK),
        )

    def op_unit(st):
        # output projection for one 128-row s-tile + its output DMA
        po = [
            psum_proj.tile([P, 512], f32, tag="proj", name=f"po{c}") for c in range(2)
        ]
        for col in range(2):
            for ot in range(2):
                nc.tensor.matmul(
                    po[col][:],
                    attnT_sb[:, ot, st * P : (st + 1) * P],
                    wo_sb[:, ot, col * 512 : (col + 1) * 512],
                    start=(ot == 0),
                    stop=(ot == 1),
                )
        ob = ostage.tile([P, DM], f32, tag="ostage")
        for col in range(2):
            nc.vector.tensor_copy(ob[:, col * 512 : (col + 1) * 512], po[col][:])
        nc.sync.dma_start(out_d[st * P : (st + 1) * P, :], ob[:])

    # ---- attention block ----------------------------------------------
    def block(ch, hp, bg):
        """one (512-q-chunk, head-pair) attention block.

        bg: dict iteration -> list of background thunks, emitted into the
        PE stream at that t-iteration (fills PE slack under the exp stream).
        """
        q0 = ch * 512
        av = [
            psum_av_pool.tile([P, 512], f32, tag="av", name=f"av{j}") for j in range(2)
        ]
        ps_tiles = {}
        ex_tiles = {}

        def scores(t):
            ps_s = psum_s_pool.tile([P, 2, 512], f32, tag="scores", name="ps_s")
            ps_tiles[t] = ps_s
            for j in range(2):
                hb = j * 64
                nc.tensor.matmul(
                    ps_s[:, j, :],
                    kT_sb[hb : hb + 64, hp, t * P : (t + 1) * P],
                    qT_sb[hb : hb + 64, hp, q0 : q0 + 512],
                    start=True,
                    stop=True,
                    tile_position=(hb, 0),
                )

        def expop(t):
            ex = exp_pool.tile([P, 2, 512], mmdt, tag="exp", name="ex")
            ex_tiles[t] = ex
            nc.scalar.activation(ex[:], ps_tiles.pop(t)[:], Exp, scale=0.125)

        def avop(t):
            ex = ex_tiles.pop(t)
            for j in range(2):
                nc.tensor.matmul(
                    av[j][0 : DK + 1, :],
                    vab_sb[:, t, 2 * hp + j, :],
                    ex[:, j, :],
                    start=(t == 0),
                    stop=(t == S_T - 1),
                )

        scores(0)
        expop(0)
        for t in range(1, S_T):
            scores(t)
            avop(t - 1)
            for fn in bg.get(t, ()):
                fn()
            expop(t)
        avop(S_T - 1)

        # epilogue: divide by softmax denominators (row DK of av).
        # fp16 1-pass broadcast matmul of each denom row across 64 partitions.
        den_b = psum_s_pool.tile([64, 2, 512], f32, tag="scores", name="den_b")
        rec = []
        for j in range(2):
            den_row = smalls.tile([1, 512], mmdt, tag="den", name=f"den{j}")
            nc.vector.tensor_copy(den_row[:], av[j][DK : DK + 1, :])
            nc.tensor.matmul(
                den_b[:, j, :], ones_sb[:], den_row[:], start=True, stop=True
            )
            rec_b = smalls.tile([64, 512], f32, tag="recb", name=f"rec{j}")
            nc.vector.reciprocal_approx_fast(rec_b[:], den_b[:, j, :])
            rec.append(rec_b)
        for j in range(2):
            lh = 2 * hp + j
            if lh % 2 == 0:
                nc.vector.tensor_mul(
                    attnT_sb[0:64, lh // 2, q0 : q0 + 512],
                    av[j][0:DK, :],
                    rec[j][:],
                )
            else:
                nc.vector.tensor_mul(
                    stage_sb[:, lh // 2, q0 : q0 + 512],
                    av[j][0:DK, :],
                    rec[j][:],
                )
                nc.sync.dma_start(
                    attnT_sb[64:128, lh // 2, q0 : q0 + 512],
                    stage_sb[:, lh // 2, q0 : q0 + 512],
                )

    # ---- schedule ------------------------------------------------------
    # prologue: minimum needed for block(0,0) t=0
    kp_unit(0, 0)
    kp_unit(0, 1)
    qp_unit(0, 0)
    qp_unit(0, 1)
    vp_unit(0)

    # block(0,0): pipeline fill — remaining K-proj chunks before their scores
    # tiles, V-proj tiles just-in-time before their AV tiles.
    block(0, 0, {
        1: [lambda: vp_unit(1), lambda: kp_unit(1, 0)],
        2: [lambda: vp_unit(2), lambda: kp_unit(1, 1)],
        3: [lambda: vp_unit(3), lambda: vp_unit(4)],
        4: [lambda: kp_unit(2, 0), lambda: vp_unit(5)],
        5: [lambda: kp_unit(2, 1), lambda: vp_unit(6)],
        6: [lambda: vp_unit(7), lambda: vp_unit(8)],
        7: [lambda: kp_unit(3, 0), lambda: vp_unit(9)],
        8: [lambda: kp_unit(3, 1), lambda: vp_unit(10)],
        9: [lambda: vp_unit(11), lambda: vp_unit(12)],
        10: [lambda: vp_unit(13)],
        11: [lambda: vp_unit(14)],
        12: [lambda: vp_unit(15)],
    })
    block(0, 1, {
        2: [lambda: qp_unit(1, 0)],
        8: [lambda: qp_unit(1, 1)],
    })
    block(1, 0, {
        2: [lambda: op_unit(0)],
        5: [lambda: op_unit(1)],
        8: [lambda: op_unit(2)],
        11: [lambda: op_unit(3)],
    })
    block(1, 1, {
        2: [lambda: qp_unit(2, 0)],
        8: [lambda: qp_unit(2, 1)],
    })
    block(2, 0, {
        2: [lambda: op_unit(4)],
        5: [lambda: op_unit(5)],
        8: [lambda: op_unit(6)],
        11: [lambda: op_unit(7)],
    })
    block(2, 1, {
        2: [lambda: qp_unit(3, 0)],
        8: [lambda: qp_unit(3, 1)],
    })
    block(3, 0, {
        2: [lambda: op_unit(8)],
        5: [lambda: op_unit(9)],
        8: [lambda: op_unit(10)],
        11: [lambda: op_unit(11)],
    })
    block(3, 1, {})
    for st in range(12, 16):
        op_unit(st)


def _get_program():
    global _PROGRAM
    if _PROGRAM is None:
        _PROGRAM = _build_program()
    return _PROGRAM


def make_in_maps(Q, K, V, W_q, W_k, W_v, W_o):
    """Per-core input dicts: core c -> batch c//4, heads (c%4)*4 ... +4."""
    mmdt = np.float16 if MM_BF16 else np.float32
    in_maps = []
    for c in range(8):
        b, g = c // 4, c % 4
        sl = slice(g * DO, (g + 1) * DO)
        in_maps.append(
            {
                "QT": np.ascontiguousarray(Q[b].T).astype(mmdt),
                "KT": np.ascontiguousarray(K[b].T).astype(mmdt),
                "VT": np.ascontiguousarray(V[b].T).astype(mmdt),
                "WQT": np.ascontiguousarray(W_q[sl, :].T).astype(mmdt),
                "WKT": np.ascontiguousarray(W_k[sl, :].T).astype(mmdt),
                "WVT": np.ascontiguousarray(W_v[sl, :].T).astype(mmdt),
                "WOT": np.ascontiguousarray(W_o[:, sl].T).astype(mmdt),
            }
        )
    return in_maps


def combine_outputs(outs):
    """outs: list of 8 [S, DM] partials -> [B, S, DM]."""
    return np.stack(
        [
            outs[0] + outs[1] + outs[2] + outs[3],
            outs[4] + outs[5] + outs[6] + outs[7],
        ]
    ).astype(np.float32)


def kernel(Q, K, V, W_q, W_k, W_v, W_o):
    from concourse.bass_utils import run_bass_kernel_spmd

    Q = np.asarray(Q)
    K = np.asarray(K)
    V = np.asarray(V)
    nc = _get_program()
    in_maps = make_in_maps(Q, K, V, np.asarray(W_q), np.asarray(W_k), np.asarray(W_v), np.asarray(W_o))
    res = run_bass_kernel_spmd(nc, in_maps, core_ids=list(range(8)))
    return combine_outputs([res.results[c]["OUT"] for c in range(8)])


# revision 5
# speedup vs baseline: 1.3813x; 1.0706x over previous
"""Multi-head attention (B=2, S=2048, D=1024, H=16) on 8 trn2 NeuronCores.

Sharding: data-parallel over batch (2 groups of 4 cores), tensor-parallel over
heads within a group (4 heads/core).  Each core computes a partial output
(its heads' contribution through its W_o column shard); the host sums the 4
partials per batch element.

Schedule (v3): the kernel is ACT-bound (exp of 4x2048x2048 scores = ~142us at
1 elem/cycle/lane @1.2GHz), so everything else is scheduled around keeping the
exp stream dense:
  - inputs are pre-tiled on the host into s-512 chunks (contiguous 4-8KB DMA
    lines) and DMAed in consumption order: the first K/Q projections start
    ~7us in and the first exp fires ~15us in.
  - all remaining projection work (V-proj, later K/Q-proj chunks, output
    projection of finished chunks) is drip-fed as background PE work INSIDE
    the attention t-loops, filling the PE slack under each exp instruction.
  - scores run TWO k-tiles ahead of AV, so a stalled AV (e.g. waiting on an
    av-psum buffer still held by the previous block's epilogue) never stalls
    the exp stream.
  - each block's av psum is drained to SBUF immediately (releasing the psum
    bank for the next block) and the softmax-normalize epilogue is deferred
    into the NEXT block's background slots.
  - the denominator broadcast matmul runs in fp16 (1-pass, not 2-pass fp32),
    and a dummy exp hoists the ~2.7us ACT table load off the critical path.

Per-core layout (unchanged from v1):
  - q, k produced TRANSPOSED ([d_local, s]); scores lhsT = kT tile, rhs = qT,
    head pairs row-packed via tile_position (0,0)/(64,0).
  - v in natural layout with a fused ones-column so the AV matmul emits the
    attention output and the softmax denominator in one accumulation chain.
  - softmax skips max-subtraction (scores ~ N(0,1); fp32 exp cannot overflow).
"""

import os
from contextlib import ExitStack

import numpy as np

B = 2
S = 2048
DM = 1024
H = 16
DK = 64
P = 128
HC = 4            # heads per core
DO = HC * DK      # 256: local output dim of q/k/v projections
DI_T = DM // P    # 8 contraction tiles for projections
S_T = S // P      # 16
N_SC = 4          # s-chunks of 512

MM_BF16 = True    # matmul inputs in fp16 (fp32 PSUM accumulation everywhere)

_PROGRAM = None


def _build_program():
    import concourse.mybir as mybir
    import concourse.tile as tile
    from concourse import bacc

    f32 = mybir.dt.float32
    mmdt = mybir.dt.float16 if MM_BF16 else f32
    nc = bacc.Bacc("TRN2", target_bir_lowering=False, debug=False)

    # pre-tiled inputs: chunk sc is contiguous ([sc][p][t][512])
    qt_d = nc.dram_tensor("QTC", [N_SC, P, DI_T, 512], mmdt, kind="ExternalInput").ap()
    kt_d = nc.dram_tensor("KTC", [N_SC, P, DI_T, 512], mmdt, kind="ExternalInput").ap()
    vt_d = nc.dram_tensor("VTC", [N_SC, P, DI_T, 512], mmdt, kind="ExternalInput").ap()
    wqt_d = nc.dram_tensor("WQC", [P, DI_T, DO], mmdt, kind="ExternalInput").ap()
    wkt_d = nc.dram_tensor("WKC", [P, DI_T, DO], mmdt, kind="ExternalInput").ap()
    wvt_d = nc.dram_tensor("WVC", [P, DI_T, DO], mmdt, kind="ExternalInput").ap()
    wot_d = nc.dram_tensor("WOC", [P, 2, DM], mmdt, kind="ExternalInput").ap()
    out_d = nc.dram_tensor("OUT", [S, DM], f32, kind="ExternalOutput").ap()

    with tile.TileContext(nc) as tc, ExitStack() as ctx:
        _emit(ctx, tc, qt_d, kt_d, vt_d, wqt_d, wkt_d, wvt_d, wot_d, out_d)
    nc.compile()
    return nc


def _emit(ctx, tc, qt_d, kt_d, vt_d, wqt_d, wkt_d, wvt_d, wot_d, out_d):
    import concourse.mybir as mybir

    nc = tc.nc
    f32 = mybir.dt.float32
    mmdt = mybir.dt.float16 if MM_BF16 else f32
    Exp = mybir.ActivationFunctionType.Exp

    consts = ctx.enter_context(tc.tile_pool(name="consts", bufs=1))
    exp_pool = ctx.enter_context(tc.tile_pool(name="exp", bufs=3))
    smalls = ctx.enter_context(tc.tile_pool(name="smalls", bufs=2))
    avdrain = ctx.enter_context(tc.tile_pool(name="avdrain", bufs=2))
    ostage = ctx.enter_context(tc.tile_pool(name="ostage", bufs=2))

    qt_sb = consts.tile([P, DI_T, S], mmdt, tag="qt")     # staged Q^T
    kt_sb = consts.tile([P, DI_T, S], mmdt, tag="kt")
    vt_sb = consts.tile([P, DI_T, S], mmdt, tag="vt")
    wq_sb = consts.tile([P, DI_T, DO], mmdt, tag="wq")
    wk_sb = consts.tile([P, DI_T, DO], mmdt, tag="wk")
    wv_sb = consts.tile([P, DI_T, DO], mmdt, tag="wv")
    wo_sb = consts.tile([P, 2, DM], mmdt, tag="wo")
    qT_sb = consts.tile([P, 2, S], mmdt, tag="qT")    # [p, ot, s]; o_local = ot*128+p
    kT_sb = consts.tile([P, 2, S], mmdt, tag="kT")
    vab_sb = consts.tile([P, S_T, HC, DK + 1], mmdt, tag="vab")
    attnT_sb = consts.tile([P, 2, S], mmdt, tag="attnT")
    stage_sb = consts.tile([64, 2, S], mmdt, tag="oddstage")
    ones_sb = consts.tile([1, 64], mmdt, tag="ones")
    warm_sb = consts.tile([P, 512], mmdt, tag="warm")

    # chunked input DMAs in consumption-priority order; kt0/qt0 gate block 0
    def in_chunk(dst, src, sc):
        nc.sync.dma_start(dst[:, :, sc * 512 : (sc + 1) * 512], src[sc])

    nc.sync.dma_start(wk_sb[:], wkt_d)
    nc.sync.dma_start(wq_sb[:], wqt_d)
    in_chunk(kt_sb, kt_d, 0)
    in_chunk(qt_sb, qt_d, 0)
    nc.sync.dma_start(wv_sb[:], wvt_d)
    in_chunk(vt_sb, vt_d, 0)
    in_chunk(kt_sb, kt_d, 1)
    in_chunk(kt_sb, kt_d, 2)
    in_chunk(kt_sb, kt_d, 3)
    in_chunk(vt_sb, vt_d, 1)
    in_chunk(vt_sb, vt_d, 2)
    in_chunk(vt_sb, vt_d, 3)
    in_chunk(qt_sb, qt_d, 1)
    in_chunk(qt_sb, qt_d, 2)
    in_chunk(qt_sb, qt_d, 3)
    nc.sync.dma_start(wo_sb[:], wot_d)

    nc.vector.memset(vab_sb[:, :, :, DK : DK + 1], 1.0)
    nc.vector.memset(ones_sb[:], 1.0)
    nc.vector.memset(warm_sb[:], 0.0)

    psum_proj = ctx.enter_context(tc.tile_pool(name="psum_proj", bufs=2, space="PSUM"))
    psum_s_pool = ctx.enter_context(tc.tile_pool(name="psum_s", bufs=2, space="PSUM"))
    psum_av_pool = ctx.enter_context(tc.tile_pool(name="psum_av", bufs=2, space="PSUM"))

    # hoist the ~2.7us exp table load off the critical path (ACT is idle here)
    dummy_ex = smalls.tile([1, 8], f32, tag="dummyex")
    nc.scalar.activation(dummy_ex[:], warm_sb[0:1, 0:8], Exp)

    # PE warmup: flips the HAM clock gate to 8/8 and covers input-DMA latency
    for i in range(16):
        wp = psum_proj.tile([P, 512], f32, tag="proj", name="warmps")
        nc.tensor.matmul(wp[:], warm_sb[:, 0:P], warm_sb[:], start=True, stop=True)

    # ---- background PE work units -------------------------------------
    def kp_unit(sc, ot, src_sb=None, wsb=None, dst=None):
        # one 512-wide s-chunk, one o-tile of a transposed projection
        src_sb = kt_sb if src_sb is None else src_sb
        wsb = wk_sb if wsb is None else wsb
        dst = kT_sb if dst is None else dst
        pp = psum_proj.tile([P, 512], f32, tag="proj", name="pp")
        for t in range(DI_T):
            nc.tensor.matmul(
                pp[:],
                wsb[:, t, ot * P : (ot + 1) * P],
                src_sb[:, t, sc * 512 : (sc + 1) * 512],
                start=(t == 0),
                stop=(t == DI_T - 1),
            )
        nc.vector.tensor_copy(dst[:, ot, sc * 512 : (sc + 1) * 512], pp[:])

    def qp_unit(sc, ot):
        kp_unit(sc, ot, src_sb=qt_sb, wsb=wq_sb, dst=qT_sb)

    def vp_unit(st):
        # V projection s-tile (natural layout, into vab; ones column preset)
        pv = psum_proj.tile([P, 256], f32, tag="proj", name="pv")
        for t in range(DI_T):
            nc.tensor.matmul(
                pv[:],
                vt_sb[:, t, st * P : (st + 1) * P],
                wv_sb[:, t, :],
                start=(t == 0),
                stop=(t == DI_T - 1),
            )
        nc.vector.tensor_copy(
            vab_sb[:, st, :, 0:DK],
            pv[:].rearrange("p (h d) -> p h d", d=DK),
        )

    def op_unit(st):
        # output projection for one 128-row s-tile + its output DMA
        po = [
            psum_proj.tile([P, 512], f32, tag="proj", name=f"po{c}") for c in range(2)
        ]
        for col in range(2):
            for ot in range(2):
                nc.tensor.matmul(
                    po[col][:],
                    attnT_sb[:, ot, st * P : (st + 1) * P],
                    wo_sb[:, ot, col * 512 : (col + 1) * 512],
                    start=(ot == 0),
                    stop=(ot == 1),
                )
        ob = ostage.tile([P, DM], f32, tag="ostage")
        for col in range(2):
            nc.vector.tensor_copy(ob[:, col * 512 : (col + 1) * 512], po[col][:])
            nc.sync.dma_start(
                out_d[st * P : (st + 1) * P, col * 512 : (col + 1) * 512],
                ob[:, col * 512 : (col + 1) * 512],
            )

    # ---- attention block ----------------------------------------------
    def block(ch, hp, bg):
        """one (512-q-chunk, head-pair) attention block.

        bg: dict iteration -> list of background thunks, emitted into the
        PE stream at that t-iteration (fills PE slack under the exp stream).
        Returns epilogue thunks (softmax normalize) to be run by the caller,
        normally deferred into the next block's bg slots.
        """
        q0 = ch * 512
        av = [
            psum_av_pool.tile([P, 512], f32, tag="av", name=f"av{j}") for j in range(2)
        ]
        ps_tiles = {}
        ex_tiles = {}

        def scores(t):
            ps_s = psum_s_pool.tile([P, 2, 512], f32, tag="scores", name="ps_s")
            ps_tiles[t] = ps_s
            for j in range(2):
                hb = j * 64
                nc.tensor.matmul(
                    ps_s[:, j, :],
                    kT_sb[hb : hb + 64, hp, t * P : (t + 1) * P],
                    qT_sb[hb : hb + 64, hp, q0 : q0 + 512],
                    start=True,
                    stop=True,
                    tile_position=(hb, 0),
                )

        def expop(t):
            ex = exp_pool.tile([P, 2, 512], mmdt, tag="exp", name="ex")
            ex_tiles[t] = ex
            nc.scalar.activation(ex[:], ps_tiles.pop(t)[:], Exp, scale=0.125)

        def avop(t):
            ex = ex_tiles.pop(t)
            for j in range(2):
                nc.tensor.matmul(
                    av[j][0 : DK + 1, :],
                    vab_sb[:, t, 2 * hp + j, :],
                    ex[:, j, :],
                    start=(t == 0),
                    stop=(t == S_T - 1),
                )

        # scores run 2 k-tiles ahead of AV so AV stalls can't starve ACT
        scores(0)
        scores(1)
        expop(0)
        for t in range(2, S_T):
            avop(t - 2)
            for fn in bg.get(t, ()):
                fn()
            scores(t)
            expop(t - 1)
        avop(S_T - 2)
        expop(S_T - 1)
        avop(S_T - 1)

        # drain av psum to SBUF immediately: releases the av banks for the
        # next block; the normalize epilogue reads the SBUF copy later.
        avs = []
        for j in range(2):
            a = avdrain.tile([DK + 1, 512], f32, tag=f"avs{j}", name=f"avs{j}")
            nc.vector.tensor_copy(a[:], av[j][0 : DK + 1, :])
            avs.append(a)

        def epi(j):
            # softmax normalize for head j: fp16 1-pass denominator broadcast
            den_row = smalls.tile([1, 512], mmdt, tag="den", name=f"den{j}")
            nc.vector.tensor_copy(den_row[:], avs[j][DK : DK + 1, :])
            den_b = psum_s_pool.tile([64, 512], f32, tag="scores", name="den_b")
            nc.tensor.matmul(den_b[:], ones_sb[:], den_row[:], start=True, stop=True)
            rec_b = smalls.tile([64, 512], f32, tag="recb", name=f"rec{j}")
            nc.vector.reciprocal_approx_fast(rec_b[:], den_b[:])
            lh = 2 * hp + j
            if lh % 2 == 0:
                nc.vector.tensor_mul(
                    attnT_sb[0:64, lh // 2, q0 : q0 + 512], avs[j][0:DK, :], rec_b[:]
                )
            else:
                nc.vector.tensor_mul(
                    stage_sb[:, lh // 2, q0 : q0 + 512], avs[j][0:DK, :], rec_b[:]
                )
                nc.sync.dma_start(
                    attnT_sb[64:128, lh // 2, q0 : q0 + 512],
                    stage_sb[:, lh // 2, q0 : q0 + 512],
                )

        return [lambda: epi(0), lambda: epi(1)]

    # ---- schedule ------------------------------------------------------
    # prologue: minimum needed for block(0,0) t=0
    kp_unit(0, 0)
    kp_unit(0, 1)
    qp_unit(0, 0)
    qp_unit(0, 1)
    vp_unit(0)

    # block(0,0): pipeline fill — remaining K-proj chunks before their scores
    # tiles, V-proj tiles just-in-time before their AV tiles.
    epi = block(0, 0, {
        2: [lambda: vp_unit(1), lambda: kp_unit(1, 0)],
        3: [lambda: vp_unit(2), lambda: kp_unit(1, 1)],
        4: [lambda: vp_unit(3), lambda: vp_unit(4)],
        5: [lambda: kp_unit(2, 0), lambda: vp_unit(5)],
        6: [lambda: kp_unit(2, 1), lambda: vp_unit(6)],
        7: [lambda: vp_unit(7), lambda: vp_unit(8)],
        8: [lambda: kp_unit(3, 0), lambda: vp_unit(9)],
        9: [lambda: kp_unit(3, 1), lambda: vp_unit(10)],
        10: [lambda: vp_unit(11), lambda: vp_unit(12)],
        11: [lambda: vp_unit(13)],
        12: [lambda: vp_unit(14)],
        13: [lambda: vp_unit(15)],
    })
    epi = block(0, 1, {
        2: [epi[0]],
        3: [epi[1]],
        5: [lambda: qp_unit(1, 0)],
        9: [lambda: qp_unit(1, 1)],
    })
    epi = block(1, 0, {
        2: [epi[0]],
        3: [epi[1]],
        5: [lambda: op_unit(0)],
        8: [lambda: op_unit(1)],
        11: [lambda: op_unit(2)],
        13: [lambda: op_unit(3)],
    })
    epi = block(1, 1, {
        2: [epi[0]],
        3: [epi[1]],
        5: [lambda: qp_unit(2, 0)],
        9: [lambda: qp_unit(2, 1)],
    })
    epi = block(2, 0, {
        2: [epi[0]],
        3: [epi[1]],
        5: [lambda: op_unit(4)],
        8: [lambda: op_unit(5)],
        11: [lambda: op_unit(6)],
        13: [lambda: op_unit(7)],
    })
    epi = block(2, 1, {
        2: [epi[0]],
        3: [epi[1]],
        5: [lambda: qp_unit(3, 0)],
        9: [lambda: qp_unit(3, 1)],
    })
    epi = block(3, 0, {
        2: [epi[0]],
        3: [epi[1]],
        5: [lambda: op_unit(8)],
        8: [lambda: op_unit(9)],
        11: [lambda: op_unit(10)],
        13: [lambda: op_unit(11)],
    })
    epi = block(3, 1, {
        2: [epi[0]],
        3: [epi[1]],
    })
    epi[0]()
    epi[1]()
    for st in range(12, 16):
        op_unit(st)


def _get_program():
    global _PROGRAM
    if _PROGRAM is None:
        _PROGRAM = _build_program()
    return _PROGRAM


def make_in_maps(Q, K, V, W_q, W_k, W_v, W_o):
    """Per-core input dicts: core c -> batch c//4, heads (c%4)*4 ... +4.

    Inputs are pre-tiled so each DMA chunk is contiguous:
      KTC[sc, p, t, s'] = K^T[t*128+p, sc*512+s']   (likewise QTC/VTC)
      WKC[p, t, o]      = W_k^T[t*128+p, o]          (likewise WQC/WVC)
      WOC[p, ot, o]     = W_o^T[ot*128+p, o]
    """
    mmdt = np.float16 if MM_BF16 else np.float32

    def tile_in(x):  # [S, DM] -> x.T pre-tiled [4, 128, 8, 512]
        return np.ascontiguousarray(
            x.T.reshape(DI_T, P, N_SC, 512).transpose(2, 1, 0, 3)
        ).astype(mmdt)

    def tile_w(w):  # [DO, DM] -> w.T pre-tiled [128, 8, 256]
        return np.ascontiguousarray(
            w.T.reshape(DI_T, P, DO).transpose(1, 0, 2)
        ).astype(mmdt)

    in_maps = []
    for c in range(8):
        b, g = c // 4, c % 4
        sl = slice(g * DO, (g + 1) * DO)
        in_maps.append(
            {
                "QTC": tile_in(Q[b]),
                "KTC": tile_in(K[b]),
                "VTC": tile_in(V[b]),
                "WQC": tile_w(W_q[sl, :]),
                "WKC": tile_w(W_k[sl, :]),
                "WVC": tile_w(W_v[sl, :]),
                "WOC": np.ascontiguousarray(
                    W_o[:, sl].T.reshape(2, P, DM).transpose(1, 0, 2)
                ).astype(mmdt),
            }
        )
    return in_maps


def combine_outputs(outs):
    """outs: list of 8 [S, DM] partials -> [B, S, DM]."""
    return np.stack(
        [
            outs[0] + outs[1] + outs[2] + outs[3],
            outs[4] + outs[5] + outs[6] + outs[7],
        ]
    ).astype(np.float32)


def kernel(Q, K, V, W_q, W_k, W_v, W_o):
    from concourse.bass_utils import run_bass_kernel_spmd

    Q = np.asarray(Q)
    K = np.asarray(K)
    V = np.asarray(V)
    nc = _get_program()
    in_maps = make_in_maps(Q, K, V, np.asarray(W_q), np.asarray(W_k), np.asarray(W_v), np.asarray(W_o))
    res = run_bass_kernel_spmd(nc, in_maps, core_ids=list(range(8)))
    return combine_outputs([res.results[c]["OUT"] for c in range(8)])
